# revision 15
# baseline (speedup 1.0000x reference)
"""FFT causal long-conv (H3/Hyena fftconv) as a blocked-Toeplitz matmul kernel
for 8 Trainium2 NeuronCores.

Math: y[b,d,l] = sum_{t<=l} filter[d,t] * x[b,d,l-t]  (causal conv, L taps).

Instead of an on-device FFT, the causal conv is computed directly as a
lower-block-triangular Toeplitz matmul: with 128-wide blocks (J=L/128 blocks),
y_i = sum_{k<=i} T_k @ x_{i-k} where T_k[a,c] = f[128k + a - c].  The T_k are
materialized host-side as PE-ready lhsT tiles, so the device does only
dense [128,128]x[128,N] matmuls accumulating in fp32 PSUM.

The dominant HBM traffic is the Toeplitz weight image (the filter replicated
128x at 1 MB/channel).  It is stored as float8e3 (E3M4, 4 mantissa bits) which
halves that traffic vs fp16; the tensor engine takes the fp8 lhsT directly
against an fp16 rhs at full bf16 rate.  The filter is pre-scaled by 64 into
e3m4's normal range and x is pre-scaled by 1/64, so no on-device dequant is
needed: (64 f) conv (x/64) = y exactly.

Sharding: channels D=1024 split 128 per core (each channel's conv is
independent); all B=16 batches stay on-core so each matmul gets the full
N=512 free dim.
"""

import numpy as np
import ml_dtypes


B, D, L = 16, 1024, 4096
NCORES = 8
DC = D // NCORES  # channels per core
C = 128           # time-block size == PE contraction dim
J = L // C        # 32 time blocks
N = J * B         # 512 = matmul free dim (j-block outer, batch inner)
GROUP = 4         # channels per DMA batch

F16 = np.float16
F8 = ml_dtypes.float8_e3m4
FSCALE = 64.0     # filter pre-scale into e3m4 normal range; x scaled by 1/64

_CACHE = {}


def _build_nc():
    if "nc" in _CACHE:
        return _CACHE["nc"]

    import concourse.bacc as bacc
    import concourse.tile as tile
    import concourse.mybir as mybir

    nc = bacc.Bacc("TRN2", target_bir_lowering=False, debug=False, num_devices=NCORES)

    # Layouts are chosen so every DMA has long contiguous per-partition runs:
    #   xt[c, d, n]    n = j*B + b         (input, time-within-block on partitions)
    #   ft[c, d, k, a] = f[d, 128k + a - c] (PE-ready lhsT Toeplitz tiles, e3m4)
    #   yt[a, d, n]    n = i*B + b         (output)
    xt = nc.dram_tensor("xt", [C, DC, N], mybir.dt.float16, kind="ExternalInput")
    ft = nc.dram_tensor("ft", [C, DC, J, C], mybir.dt.float8e3, kind="ExternalInput")
    yt = nc.dram_tensor("yt", [C, DC, N], mybir.dt.float16, kind="ExternalOutput")

    with tile.TileContext(nc) as tc:
        with (
            tc.tile_pool(name="wpool", bufs=3) as wpool,
            tc.tile_pool(name="xpool", bufs=3) as xpool,
            tc.tile_pool(name="ypool", bufs=3) as ypool,
            tc.tile_pool(name="pspool", bufs=7, space="PSUM") as pspool,
            tc.tile_pool(name="warmps", bufs=1, space="PSUM") as warmps,
        ):
            # The PE otherwise idles waiting for the first weight DMA and then
            # pays the HAM half-clock ramp (~3.4us of sustained activity to
            # trip K=8/8). A few dummy matmuls bridge until group 0's first
            # per-channel weight slice lands (~7us); the first real chains
            # finish the ramp as useful work.
            wz = wpool.tile([C, C], mybir.dt.float16, tag="warmz", bufs=1)
            nc.vector.memset(wz, 0.0)
            # 64 dummy matmuls x ~58ns cold = ~3.7us of sustained PE activity
            # ending right when the first weight/x semaphores fire (~10.7us,
            # the end-to-end DMA latency floor) -- trips the HAM clock gate to
            # K=8/8 so the real matmuls start at full 2.4 GHz.
            wps = warmps.tile([C, N], mybir.dt.float32)
            for _ in range(64):
                nc.tensor.matmul(wps[:, :64], wz[:, :C], wz[:, :64],
                                 start=True, stop=True)
            NG = DC // GROUP
            for g in range(NG):
                sl = slice(g * GROUP, (g + 1) * GROUP)
                # Keep both HWDGE rings (SP + ACT) continuously busy: each
                # group's weight load is split half/half across the rings.
                # Everything stays off the slow gpsimd SWDGE path.
                eng_a = nc.sync if g % 2 == 0 else nc.scalar
                eng_b = nc.scalar if g % 2 == 0 else nc.sync
                xg = xpool.tile([C, GROUP, N], mybir.dt.float16)
                wt = wpool.tile([C, GROUP, J, C], mybir.dt.float8e3)
                if g == 0:
                    # Finest-grained first loads, spread across both rings:
                    # channel 0's x (128 KB) and the k<16 half of its weights
                    # (256 KB) gate the first real matmul; everything else
                    # streams in behind them.
                    eng_b.dma_start(out=xg[:, :1], in_=xt[:, :1, :])
                    eng_a.dma_start(out=wt[:, :1, :J // 2], in_=ft[:, :1, :J // 2, :])
                    eng_a.dma_start(out=wt[:, :1, J // 2:], in_=ft[:, :1, J // 2:, :])
                    eng_b.dma_start(out=xg[:, 1:], in_=xt[:, 1:GROUP, :])
                    for dd in range(1, GROUP):
                        eng = eng_a if dd % 2 == 0 else eng_b
                        eng.dma_start(out=wt[:, dd:dd + 1],
                                      in_=ft[:, dd:dd + 1, :, :])
                else:
                    eng_b.dma_start(out=xg, in_=xt[:, sl, :])
                    h = GROUP // 2
                    eng_a.dma_start(out=wt[:, :h], in_=ft[:, sl.start:sl.start + h, :, :])
                    eng_b.dma_start(out=wt[:, h:], in_=ft[:, sl.start + h:sl.stop, :, :])
                yg = ypool.tile([C, GROUP, N], mybir.dt.float16)
                for dd in range(GROUP):
                    ps = pspool.tile([C, N], mybir.dt.float32)
                    last = g == NG - 1 and dd == GROUP - 1
                    for k in range(J):
                        ncols = (J - k) * B
                        nc.tensor.matmul(
                            ps[:, k * B:],
                            wt[:, dd, k, :],
                            xg[:, dd, :ncols],
                            # For the final channel the chain is split into
                            # two accumulation groups at k=16: matmuls k>=16
                            # only touch cols [256,512), so ending a group at
                            # k=15 lets the cols [0,256) drain run during the
                            # tail matmuls instead of after them.
                            start=(k == 0),
                            stop=(k == J - 1 or (last and k == 15)),
                            skip_group_check=last and k >= 16,
                        )
                        if last and k == 15:
                            nc.vector.tensor_copy(out=yg[:, dd, :C * 2],
                                                  in_=ps[:, :C * 2])
                            eng_b.dma_start(out=yt[:, sl.start + dd, :C * 2],
                                            in_=yg[:, dd, :C * 2])
                    if last:
                        nc.vector.tensor_copy(out=yg[:, dd, C * 2:],
                                              in_=ps[:, C * 2:])
                        eng_b.dma_start(out=yt[:, sl.start + dd, C * 2:],
                                        in_=yg[:, dd, C * 2:])
                    else:
                        nc.vector.tensor_copy(out=yg[:, dd, :], in_=ps[:])
                        # Store each channel as soon as its PSUM drain
                        # finishes so the final store isn't serialized behind
                        # the whole group.
                        eng_b.dma_start(out=yt[:, sl.start + dd, :],
                                        in_=yg[:, dd, :])

    nc.compile()
    _CACHE["nc"] = nc
    return nc


def _prep_core_inputs(x, f, core):
    ds = slice(core * DC, (core + 1) * DC)
    xs = x[:, ds, :].reshape(B, DC, J, C).transpose(3, 1, 2, 0).reshape(C, DC, N)
    xt = np.ascontiguousarray(xs * (1.0 / FSCALE)).astype(F16)

    # Convert the (scaled) filter to e3m4 FIRST, then build the 128x
    # replicated Toeplitz image as a byte-level strided copy.
    # fpad[d, 127 + t] = f[d, t]; ft[c, d, m] = fpad[d, 127 + m - c]
    fpad = np.zeros((DC, 127 + L), dtype=F8)
    fpad[:, 127:] = (f[ds] * FSCALE).astype(F8)
    base = fpad[:, 127:]
    sv = np.lib.stride_tricks.as_strided(
        base,
        shape=(C, DC, L),
        strides=(-fpad.strides[1], fpad.strides[0], fpad.strides[1]),
    )
    ft = np.ascontiguousarray(sv).reshape(C, DC, J, C)
    return {"xt": xt, "ft": ft}


def _run(x, f, trace=False):
    from concourse.bass_utils import run_bass_kernel_spmd

    nc = _build_nc()
    in_maps = [_prep_core_inputs(x, f, i) for i in range(NCORES)]
    res = run_bass_kernel_spmd(
        nc, in_maps, core_ids=list(range(NCORES)), trace=trace
    )

    y = np.empty((B, D, L), dtype=np.float32)
    for i in range(NCORES):
        ytc = np.asarray(res.results[i]["yt"]).astype(np.float32)  # [C(a), DC, N]
        ys = ytc.reshape(C, DC, J, B).transpose(3, 1, 2, 0).reshape(B, DC, L)
        y[:, i * DC:(i + 1) * DC, :] = ys
    return y, res


def kernel(x, filter):
    x = np.asarray(x, dtype=np.float32)
    f = np.asarray(filter, dtype=np.float32)
    y, _ = _run(x, f, trace=False)
    return y


# revision 21
# speedup vs baseline: 1.2170x; 1.2170x over previous
"""FFT causal long-conv (H3/Hyena fftconv) as a blocked-Toeplitz matmul kernel
for 8 Trainium2 NeuronCores.

Math: y[b,d,l] = sum_{t<=l} filter[d,t] * x[b,d,l-t]  (causal conv, L taps).

Instead of an on-device FFT, the causal conv is computed directly as a
lower-block-triangular Toeplitz matmul: with 128-wide blocks (J=L/128 blocks),
y_i = sum_{k<=i} T_k @ x_{i-k} where T_k[a,c] = f[128k + a - c].  The T_k are
materialized host-side as PE-ready lhsT tiles, so the device does only
dense [128,128]x[128,N] matmuls accumulating in fp32 PSUM.

The dominant HBM traffic is the Toeplitz weight image (the filter replicated
128x at 1 MB/channel).  It is stored as float8e3 (E3M4, 4 mantissa bits) which
halves that traffic vs fp16; the tensor engine takes the fp8 lhsT directly
against an fp16 rhs at full bf16 rate.  The filter is pre-scaled by 64 into
e3m4's normal range and x is pre-scaled by 1/64, so no on-device dequant is
needed: (64 f) conv (x/64) = y exactly.

Sharding: channels D=1024 split 128 per core (each channel's conv is
independent); all B=16 batches stay on-core so each matmul gets the full
N=512 free dim.
"""

import numpy as np
import ml_dtypes


B, D, L = 16, 1024, 4096
NCORES = 8
DC = D // NCORES  # channels per core
C = 128           # time-block size == PE contraction dim
J = L // C        # 32 time blocks
N = J * B         # 512 = matmul free dim (j-block outer, batch inner)
GROUP = 4         # channels per DMA batch

F16 = np.float16
F8 = ml_dtypes.float8_e3m4
F8E4 = ml_dtypes.float8_e4m3
FSCALE = 64.0     # filter pre-scale into e3m4 normal range; x scaled by 1/64

# Tap-block pair (k=14,15) computed via a DoubleRow fp8e4 matmul: two weight
# tiles interleaved in one stationary, two rhs columns consumed per output
# column -- halves the PE time for those two tiles.  Operands use their own
# scales (f*16, x*32, both centered in e4m3's normal range) and accumulate in
# a separate PSUM bank that is rescaled by 2^-9 when combined at drain time.
DRK = 14
NP = (J - DRK) * B            # 288 DoubleRow output columns
DR_COMBINE = 2.0 ** -9        # (16 * 32) ^ -1

_CACHE = {}


def _build_nc():
    if "nc" in _CACHE:
        return _CACHE["nc"]

    import concourse.bacc as bacc
    import concourse.tile as tile
    import concourse.mybir as mybir

    nc = bacc.Bacc("TRN2", target_bir_lowering=False, debug=False, num_devices=NCORES)

    # Layouts are chosen so every DMA has long contiguous per-partition runs:
    #   xt[c, d, n]    n = j*B + b         (input, time-within-block on partitions)
    #   ft[c, d, k, a] = f[d, 128k + a - c] (PE-ready lhsT Toeplitz tiles, e3m4)
    #   yt[a, d, n]    n = i*B + b         (output)
    xt = nc.dram_tensor("xt", [C, DC, N], mybir.dt.float16, kind="ExternalInput")
    ft = nc.dram_tensor("ft", [C, DC, J, C], mybir.dt.float8e3, kind="ExternalInput")
    ftp = nc.dram_tensor("ftp", [C, DC, 2, C], mybir.dt.float8e4, kind="ExternalInput")
    xp = nc.dram_tensor("xp", [C, DC, 2, NP], mybir.dt.float8e4, kind="ExternalInput")
    yt = nc.dram_tensor("yt", [C, DC, N], mybir.dt.float16, kind="ExternalOutput")

    with tile.TileContext(nc) as tc:
        with (
            tc.tile_pool(name="wpool", bufs=3) as wpool,
            tc.tile_pool(name="xpool", bufs=3) as xpool,
            tc.tile_pool(name="ypool", bufs=3) as ypool,
            tc.tile_pool(name="pspool", bufs=6, space="PSUM") as pspool,
            tc.tile_pool(name="psbpool", bufs=1, space="PSUM") as psbpool,
            tc.tile_pool(name="warmps", bufs=1, space="PSUM") as warmps,
        ):
            # The PE otherwise idles waiting for the first weight DMA and then
            # pays the HAM half-clock ramp (~3.4us of sustained activity to
            # trip K=8/8). A few dummy matmuls bridge until group 0's first
            # per-channel weight slice lands (~7us); the first real chains
            # finish the ramp as useful work.
            wz = wpool.tile([C, C], mybir.dt.float16, tag="warmz", bufs=1)
            nc.vector.memset(wz, 0.0)
            # 64 dummy matmuls x ~58ns cold = ~3.7us of sustained PE activity
            # ending right when the first weight/x semaphores fire (~10.7us,
            # the end-to-end DMA latency floor) -- trips the HAM clock gate to
            # K=8/8 so the real matmuls start at full 2.4 GHz.
            wps = warmps.tile([C, N], mybir.dt.float32)
            for _ in range(64):
                nc.tensor.matmul(wps[:, :64], wz[:, :C], wz[:, :64],
                                 start=True, stop=True)
            NG = DC // GROUP
            for g in range(NG):
                sl = slice(g * GROUP, (g + 1) * GROUP)
                # Keep both HWDGE rings (SP + ACT) continuously busy: each
                # group's weight load is split half/half across the rings.
                # Everything stays off the slow gpsimd SWDGE path.
                eng_a = nc.sync if g % 2 == 0 else nc.scalar
                eng_b = nc.scalar if g % 2 == 0 else nc.sync
                xg = xpool.tile([C, GROUP, N], mybir.dt.float16)
                wt = wpool.tile([C, GROUP, J, C], mybir.dt.float8e3)
                wtp = wpool.tile([C, GROUP, 2, C], mybir.dt.float8e4, tag="wtp")
                xpg = xpool.tile([C, GROUP, 2, NP], mybir.dt.float8e4, tag="xpg")
                if g == 0:
                    # Finest-grained first loads, spread across both rings:
                    # channel 0's x (128 KB) and the k<16 half of its weights
                    # (256 KB) gate the first real matmul; everything else
                    # streams in behind them.
                    eng_b.dma_start(out=xg[:, :1], in_=xt[:, :1, :])
                    eng_a.dma_start(out=wt[:, :1, :J // 2], in_=ft[:, :1, :J // 2, :])
                    eng_a.dma_start(out=wt[:, :1, J // 2:], in_=ft[:, :1, J // 2:, :])
                    eng_a.dma_start(out=wtp, in_=ftp[:, sl, :, :])
                    eng_b.dma_start(out=xpg, in_=xp[:, sl, :, :])
                    eng_b.dma_start(out=wt[:, 1:2], in_=ft[:, 1:2, :, :])
                    eng_b.dma_start(out=xg[:, 1:], in_=xt[:, 1:GROUP, :])
                    eng_a.dma_start(out=wt[:, 2:3], in_=ft[:, 2:3, :, :])
                    eng_b.dma_start(out=wt[:, 3:4], in_=ft[:, 3:4, :, :])
                else:
                    eng_b.dma_start(out=xg, in_=xt[:, sl, :])
                    h = GROUP // 2
                    eng_a.dma_start(out=wt[:, :h], in_=ft[:, sl.start:sl.start + h, :, :])
                    eng_b.dma_start(out=wt[:, h:], in_=ft[:, sl.start + h:sl.stop, :, :])
                    eng_a.dma_start(out=wtp, in_=ftp[:, sl, :, :])
                    eng_b.dma_start(out=xpg, in_=xp[:, sl, :, :])
                yg = ypool.tile([C, GROUP, N], mybir.dt.float16)
                LO = DRK * B  # 224: cols below the DoubleRow region
                for dd in range(GROUP):
                    ps = pspool.tile([C, N], mybir.dt.float32)
                    last = g == NG - 1 and dd == GROUP - 1
                    for k in range(DRK):
                        ncols = (J - k) * B
                        nc.tensor.matmul(
                            ps[:, k * B:],
                            wt[:, dd, k, :],
                            xg[:, dd, :ncols],
                            # For the final channel the chain is split into
                            # two accumulation groups at k=13: matmuls k>=16
                            # only touch cols [256,512), so the cols [0,224)
                            # drain runs during the tail matmuls instead of
                            # after them.
                            start=(k == 0),
                            stop=(last and k == DRK - 1),
                        )
                    if last:
                        nc.vector.tensor_copy(out=yg[:, dd, :LO],
                                              in_=ps[:, :LO])
                        eng_b.dma_start(out=yt[:, sl.start + dd, :LO],
                                        in_=yg[:, dd, :LO])
                    psb = psbpool.tile([C, NP], mybir.dt.float32)
                    nc.tensor.matmul(
                        psb,
                        wtp[:, dd],
                        xpg[:, dd],
                        start=True,
                        stop=True,
                        perf_mode=mybir.MatmulPerfMode.DoubleRow,
                        skip_group_check=True,
                    )
                    for k in range(DRK + 2, J):
                        ncols = (J - k) * B
                        nc.tensor.matmul(
                            ps[:, k * B:],
                            wt[:, dd, k, :],
                            xg[:, dd, :ncols],
                            start=False,
                            stop=(k == J - 1),
                            skip_group_check=True,
                        )
                    if last:
                        nc.vector.tensor_copy(out=yg[:, dd, LO:],
                                              in_=ps[:, LO:])
                    else:
                        nc.vector.tensor_copy(out=yg[:, dd, :], in_=ps[:])
                    # y[224:512) += 2^-9 * psb (DVE reads at most one PSUM
                    # input, so the main PSUM is cast to SBUF first and the
                    # fused multiply-add runs in place on the fp16 tile).
                    nc.vector.scalar_tensor_tensor(
                        out=yg[:, dd, LO:],
                        in0=psb,
                        scalar=DR_COMBINE,
                        in1=yg[:, dd, LO:],
                        op0=mybir.AluOpType.mult,
                        op1=mybir.AluOpType.add,
                    )
                    if last:
                        eng_b.dma_start(out=yt[:, sl.start + dd, LO:],
                                        in_=yg[:, dd, LO:])
                    else:
                        # Store each channel as soon as its PSUM drain
                        # finishes so the final store isn't serialized behind
                        # the whole group.
                        eng_b.dma_start(out=yt[:, sl.start + dd, :],
                                        in_=yg[:, dd, :])

    nc.compile()
    _CACHE["nc"] = nc
    return nc


def _prep_core_inputs(x, f, core):
    ds = slice(core * DC, (core + 1) * DC)
    xb = x[:, ds, :].reshape(B, DC, J, C).transpose(3, 1, 2, 0)  # [C, DC, J, B]
    xt = np.ascontiguousarray(xb.reshape(C, DC, N) * (1.0 / FSCALE)).astype(F16)

    # Convert the (scaled) filter to e3m4 FIRST, then build the 128x
    # replicated Toeplitz image as a byte-level strided copy.
    # fpad[d, 127 + t] = f[d, t]; ft[c, d, m] = fpad[d, 127 + m - c]
    fpad = np.zeros((DC, 127 + L), dtype=F8)
    fpad[:, 127:] = (f[ds] * FSCALE).astype(F8)
    base = fpad[:, 127:]
    sv = np.lib.stride_tricks.as_strided(
        base,
        shape=(C, DC, L),
        strides=(-fpad.strides[1], fpad.strides[0], fpad.strides[1]),
    )
    ft = np.ascontiguousarray(sv).reshape(C, DC, J, C)

    # DoubleRow pair (k=14,15): e4m3 tiles from f*16 and e4m3 x*32 image
    # with the (x_j, x_{j-1}) columns interleaved on a separate axis.
    fpad4 = np.zeros((DC, 127 + L), dtype=F8E4)
    fpad4[:, 127:] = (f[ds] * 16.0).astype(F8E4)
    base4 = fpad4[:, 127:]
    s = fpad4.strides
    ftp = np.empty((C, DC, 2, C), dtype=F8E4)
    for ko in range(2):
        sv4 = np.lib.stride_tricks.as_strided(
            base4[:, 128 * (DRK + ko):],
            shape=(C, DC, C),
            strides=(-s[1], s[0], s[1]),
        )
        ftp[:, :, ko, :] = sv4
    xq = (xb * 32.0).astype(F8E4)  # [C, DC, J, B]
    nj = J - DRK
    xp = np.zeros((C, DC, 2, nj, B), dtype=F8E4)
    xp[:, :, 0] = xq[:, :, :nj]
    xp[:, :, 1, 1:] = xq[:, :, :nj - 1]
    xp = xp.reshape(C, DC, 2, NP)
    return {"xt": xt, "ft": ft, "ftp": np.ascontiguousarray(ftp), "xp": xp}


def _run(x, f, trace=False):
    from concourse.bass_utils import run_bass_kernel_spmd

    nc = _build_nc()
    in_maps = [_prep_core_inputs(x, f, i) for i in range(NCORES)]
    res = run_bass_kernel_spmd(
        nc, in_maps, core_ids=list(range(NCORES)), trace=trace
    )

    y = np.empty((B, D, L), dtype=np.float32)
    for i in range(NCORES):
        ytc = np.asarray(res.results[i]["yt"]).astype(np.float32)  # [C(a), DC, N]
        ys = ytc.reshape(C, DC, J, B).transpose(3, 1, 2, 0).reshape(B, DC, L)
        y[:, i * DC:(i + 1) * DC, :] = ys
    return y, res


def kernel(x, filter):
    x = np.asarray(x, dtype=np.float32)
    f = np.asarray(filter, dtype=np.float32)
    y, _ = _run(x, f, trace=False)
    return y


# revision 26
# speedup vs baseline: 1.2222x; 1.0042x over previous
"""FFT causal long-conv (H3/Hyena fftconv) as a blocked-Toeplitz matmul kernel
for 8 Trainium2 NeuronCores.

Math: y[b,d,l] = sum_{t<=l} filter[d,t] * x[b,d,l-t]  (causal conv, L taps).

Instead of an on-device FFT, the causal conv is computed directly as a
lower-block-triangular Toeplitz matmul: with 128-wide blocks (J=L/128 blocks),
y_i = sum_{k<=i} T_k @ x_{i-k} where T_k[a,c] = f[128k + a - c].  The T_k are
materialized host-side as PE-ready lhsT tiles, so the device does only
dense [128,128]x[128,N] matmuls accumulating in fp32 PSUM.

The dominant HBM traffic is the Toeplitz weight image (the filter replicated
128x at 1 MB/channel).  It is stored as float8e3 (E3M4, 4 mantissa bits) which
halves that traffic vs fp16; the tensor engine takes the fp8 lhsT directly
against an fp16 rhs at full bf16 rate.  The filter is pre-scaled by 64 into
e3m4's normal range and x is pre-scaled by 1/64, so no on-device dequant is
needed: (64 f) conv (x/64) = y exactly.

Sharding: channels D=1024 split 128 per core (each channel's conv is
independent); all B=16 batches stay on-core so each matmul gets the full
N=512 free dim.
"""

import numpy as np
import ml_dtypes


B, D, L = 16, 1024, 4096
NCORES = 8
DC = D // NCORES  # channels per core
C = 128           # time-block size == PE contraction dim
J = L // C        # 32 time blocks
N = J * B         # 512 = matmul free dim (j-block outer, batch inner)
GROUP = 4         # channels per DMA batch

F16 = np.float16
F8 = ml_dtypes.float8_e3m4
F8E4 = ml_dtypes.float8_e4m3
FSCALE = 64.0     # filter pre-scale into e3m4 normal range; x scaled by 1/64

# Tap-block pairs (k=14,15) and (k=16,17) computed via DoubleRow fp8e4
# matmuls: two weight tiles interleaved in one stationary, two rhs columns
# consumed per output column -- halves the PE time for those four tiles.
# Operands use their own scales (f*16, x*32, both centered in e4m3's normal
# range) and accumulate in a separate PSUM bank that is rescaled by 2^-9 when
# combined at drain time.  To pay for the second pair's extra e4m3 noise, the
# ten highest-MAC tiles (k=0..9) use fp16 weights instead of e3m4 -- DMA has
# the headroom and the tensor engine runs fp16 lhsT at the same rate.
DRK = 14
DRK2 = 16
NP = (J - DRK) * B            # 288 DoubleRow-region output columns
NP2 = (J - DRK2) * B          # 256 for the second pair
DR_COMBINE = 2.0 ** -9        # (16 * 32) ^ -1
KF16 = 10                     # tiles k < KF16 use fp16 weights
# e3m4 tile list: k = 10..13 and 18..31 (DR tiles excluded)
K8 = [k for k in range(KF16, J) if not (DRK <= k < DRK2 + 2)]

_CACHE = {}


def _build_nc():
    if "nc" in _CACHE:
        return _CACHE["nc"]

    import concourse.bacc as bacc
    import concourse.tile as tile
    import concourse.mybir as mybir

    nc = bacc.Bacc("TRN2", target_bir_lowering=False, debug=False, num_devices=NCORES)

    # Layouts are chosen so every DMA has long contiguous per-partition runs:
    #   xt[c, d, n]    n = j*B + b         (input, time-within-block on partitions)
    #   ft[c, d, k, a] = f[d, 128k + a - c] (PE-ready lhsT Toeplitz tiles, e3m4)
    #   yt[a, d, n]    n = i*B + b         (output)
    xt = nc.dram_tensor("xt", [C, DC, N], mybir.dt.float16, kind="ExternalInput")
    ft = nc.dram_tensor("ft", [C, DC, len(K8), C], mybir.dt.float8e3, kind="ExternalInput")
    ft16 = nc.dram_tensor("ft16", [C, DC, KF16, C], mybir.dt.float16, kind="ExternalInput")
    ftp = nc.dram_tensor("ftp", [C, DC, 2, C], mybir.dt.float8e4, kind="ExternalInput")
    ftp2 = nc.dram_tensor("ftp2", [C, DC, 2, C], mybir.dt.float8e4, kind="ExternalInput")
    xp = nc.dram_tensor("xp", [C, DC, 2, NP], mybir.dt.float8e4, kind="ExternalInput")
    xp2 = nc.dram_tensor("xp2", [C, DC, 2, NP2], mybir.dt.float8e4, kind="ExternalInput")
    yt = nc.dram_tensor("yt", [C, DC, N], mybir.dt.float16, kind="ExternalOutput")

    with tile.TileContext(nc) as tc:
        with (
            tc.tile_pool(name="wpool", bufs=3) as wpool,
            tc.tile_pool(name="xpool", bufs=3) as xpool,
            tc.tile_pool(name="ypool", bufs=3) as ypool,
            tc.tile_pool(name="pspool", bufs=6, space="PSUM") as pspool,
            tc.tile_pool(name="psbpool", bufs=1, space="PSUM") as psbpool,
            tc.tile_pool(name="warmps", bufs=1, space="PSUM") as warmps,
        ):
            # The PE otherwise idles waiting for the first weight DMA and then
            # pays the HAM half-clock ramp (~3.4us of sustained activity to
            # trip K=8/8). A few dummy matmuls bridge until group 0's first
            # per-channel weight slice lands (~7us); the first real chains
            # finish the ramp as useful work.
            wz = wpool.tile([C, C], mybir.dt.float16, tag="warmz", bufs=1)
            nc.vector.memset(wz, 0.0)
            # 64 dummy matmuls x ~58ns cold = ~3.7us of sustained PE activity
            # ending right when the first weight/x semaphores fire (~10.7us,
            # the end-to-end DMA latency floor) -- trips the HAM clock gate to
            # K=8/8 so the real matmuls start at full 2.4 GHz.
            wps = warmps.tile([C, N], mybir.dt.float32)
            for _ in range(64):
                nc.tensor.matmul(wps[:, :64], wz[:, :C], wz[:, :64],
                                 start=True, stop=True)
            NG = DC // GROUP
            for g in range(NG):
                sl = slice(g * GROUP, (g + 1) * GROUP)
                # Keep both HWDGE rings (SP + ACT) continuously busy: each
                # group's weight load is split half/half across the rings.
                # Everything stays off the slow gpsimd SWDGE path.
                eng_a = nc.sync if g % 2 == 0 else nc.scalar
                eng_b = nc.scalar if g % 2 == 0 else nc.sync
                xg = xpool.tile([C, GROUP, N], mybir.dt.float16)
                wt = wpool.tile([C, GROUP, len(K8), C], mybir.dt.float8e3)
                wt16 = wpool.tile([C, GROUP, KF16, C], mybir.dt.float16, tag="wt16")
                wtp = wpool.tile([C, GROUP, 2, C], mybir.dt.float8e4, tag="wtp")
                wtp2 = wpool.tile([C, GROUP, 2, C], mybir.dt.float8e4, tag="wtp2")
                xpg = xpool.tile([C, GROUP, 2, NP], mybir.dt.float8e4, tag="xpg")
                xpg2 = xpool.tile([C, GROUP, 2, NP2], mybir.dt.float8e4, tag="xpg2")
                if g == 0:
                    # Finest-grained first loads, spread across both rings:
                    # channel 0's x (128 KB) and its k<10 fp16 weights gate
                    # the first real matmul; everything else streams behind.
                    eng_b.dma_start(out=xg[:, :1], in_=xt[:, :1, :])
                    eng_a.dma_start(out=wt16[:, :1], in_=ft16[:, :1, :, :])
                    eng_a.dma_start(out=wt[:, :1], in_=ft[:, :1, :, :])
                    eng_b.dma_start(out=xpg, in_=xp[:, sl, :, :])
                    eng_b.dma_start(out=xpg2, in_=xp2[:, sl, :, :])
                    eng_b.dma_start(out=wtp, in_=ftp[:, sl, :, :])
                    eng_b.dma_start(out=wtp2, in_=ftp2[:, sl, :, :])
                    eng_b.dma_start(out=xg[:, 1:], in_=xt[:, 1:GROUP, :])
                    eng_a.dma_start(out=wt16[:, 1:], in_=ft16[:, 1:GROUP, :, :])
                    eng_a.dma_start(out=wt[:, 1:], in_=ft[:, 1:GROUP, :, :])
                else:
                    eng_b.dma_start(out=xg, in_=xt[:, sl, :])
                    eng_a.dma_start(out=wt16, in_=ft16[:, sl, :, :])
                    eng_a.dma_start(out=wt, in_=ft[:, sl, :, :])
                    eng_a.dma_start(out=wtp, in_=ftp[:, sl, :, :])
                    eng_b.dma_start(out=wtp2, in_=ftp2[:, sl, :, :])
                    eng_b.dma_start(out=xpg, in_=xp[:, sl, :, :])
                    eng_b.dma_start(out=xpg2, in_=xp2[:, sl, :, :])
                yg = ypool.tile([C, GROUP, N], mybir.dt.float16)
                LO = DRK * B  # 224: cols below the DoubleRow region
                for dd in range(GROUP):
                    ps = pspool.tile([C, N], mybir.dt.float32)
                    last = g == NG - 1 and dd == GROUP - 1
                    for k in range(DRK):
                        ncols = (J - k) * B
                        lhs = (wt16[:, dd, k, :] if k < KF16
                               else wt[:, dd, k - KF16, :])
                        nc.tensor.matmul(
                            ps[:, k * B:],
                            lhs,
                            xg[:, dd, :ncols],
                            # For the final channel the chain is split into
                            # two accumulation groups at k=13: matmuls k>=18
                            # only touch cols [288,512), so the cols [0,224)
                            # drain runs during the tail matmuls instead of
                            # after them.
                            start=(k == 0),
                            stop=(last and k == DRK - 1),
                        )
                    if last:
                        nc.vector.tensor_copy(out=yg[:, dd, :LO],
                                              in_=ps[:, :LO])
                        eng_b.dma_start(out=yt[:, sl.start + dd, :LO],
                                        in_=yg[:, dd, :LO])
                    psb = psbpool.tile([C, NP], mybir.dt.float32)
                    nc.tensor.matmul(
                        psb,
                        wtp[:, dd],
                        xpg[:, dd],
                        start=True,
                        stop=False,
                        perf_mode=mybir.MatmulPerfMode.DoubleRow,
                        skip_group_check=True,
                    )
                    nc.tensor.matmul(
                        psb[:, 2 * B:],
                        wtp2[:, dd],
                        xpg2[:, dd],
                        start=False,
                        stop=True,
                        perf_mode=mybir.MatmulPerfMode.DoubleRow,
                        skip_group_check=True,
                    )
                    for k in range(DRK2 + 2, J):
                        ncols = (J - k) * B
                        nc.tensor.matmul(
                            ps[:, k * B:],
                            wt[:, dd, k - DRK2 - 2 + 4, :],
                            xg[:, dd, :ncols],
                            start=False,
                            stop=(k == J - 1),
                            skip_group_check=True,
                        )
                    if last:
                        nc.vector.tensor_copy(out=yg[:, dd, LO:],
                                              in_=ps[:, LO:])
                    else:
                        nc.vector.tensor_copy(out=yg[:, dd, :], in_=ps[:])
                    # y[224:512) += 2^-9 * psb (DVE reads at most one PSUM
                    # input, so the main PSUM is cast to SBUF first and the
                    # fused multiply-add runs in place on the fp16 tile).
                    nc.vector.scalar_tensor_tensor(
                        out=yg[:, dd, LO:],
                        in0=psb,
                        scalar=DR_COMBINE,
                        in1=yg[:, dd, LO:],
                        op0=mybir.AluOpType.mult,
                        op1=mybir.AluOpType.add,
                    )
                    if last:
                        eng_b.dma_start(out=yt[:, sl.start + dd, LO:],
                                        in_=yg[:, dd, LO:])
                    else:
                        # Store each channel as soon as its PSUM drain
                        # finishes so the final store isn't serialized behind
                        # the whole group.
                        eng_b.dma_start(out=yt[:, sl.start + dd, :],
                                        in_=yg[:, dd, :])

    nc.compile()
    _CACHE["nc"] = nc
    return nc


def _prep_core_inputs(x, f, core):
    ds = slice(core * DC, (core + 1) * DC)
    xb = x[:, ds, :].reshape(B, DC, J, C).transpose(3, 1, 2, 0)  # [C, DC, J, B]
    xt = np.ascontiguousarray(xb.reshape(C, DC, N) * (1.0 / FSCALE)).astype(F16)

    # Convert the (scaled) filter to e3m4 FIRST, then build the 128x
    # replicated Toeplitz image as a byte-level strided copy.
    # fpad[d, 127 + t] = f[d, t]; ft[c, d, m] = fpad[d, 127 + m - c]
    fpad = np.zeros((DC, 127 + L), dtype=F8)
    fpad[:, 127:] = (f[ds] * FSCALE).astype(F8)
    base = fpad[:, 127:]
    sv = np.lib.stride_tricks.as_strided(
        base,
        shape=(C, DC, L),
        strides=(-fpad.strides[1], fpad.strides[0], fpad.strides[1]),
    ).reshape(C, DC, J, C)
    ft = np.ascontiguousarray(sv[:, :, K8, :])

    # fp16 weights for the highest-MAC tiles k < KF16 (same f*64 scale).
    fpad16 = np.zeros((DC, 127 + L), dtype=F16)
    fpad16[:, 127:] = (f[ds] * FSCALE).astype(F16)
    b16 = fpad16[:, 127:]
    sv16 = np.lib.stride_tricks.as_strided(
        b16,
        shape=(C, DC, L),
        strides=(-fpad16.strides[1], fpad16.strides[0], fpad16.strides[1]),
    ).reshape(C, DC, J, C)
    ft16 = np.ascontiguousarray(sv16[:, :, :KF16, :])

    # DoubleRow pairs (k=14,15) and (k=16,17): e4m3 tiles from f*16 and an
    # e4m3 x*32 image with (x_j, x_{j-1}) columns interleaved on the Ko axis.
    fpad4 = np.zeros((DC, 127 + L), dtype=F8E4)
    fpad4[:, 127:] = (f[ds] * 16.0).astype(F8E4)
    base4 = fpad4[:, 127:]
    s = fpad4.strides
    ftp = np.empty((C, DC, 2, C), dtype=F8E4)
    ftp2 = np.empty((C, DC, 2, C), dtype=F8E4)
    for ko in range(2):
        for k0, dst in ((DRK, ftp), (DRK2, ftp2)):
            sv4 = np.lib.stride_tricks.as_strided(
                base4[:, 128 * (k0 + ko):],
                shape=(C, DC, C),
                strides=(-s[1], s[0], s[1]),
            )
            dst[:, :, ko, :] = sv4
    xq = (xb * 32.0).astype(F8E4)  # [C, DC, J, B]
    nj = J - DRK
    xp = np.zeros((C, DC, 2, nj, B), dtype=F8E4)
    xp[:, :, 0] = xq[:, :, :nj]
    xp[:, :, 1, 1:] = xq[:, :, :nj - 1]
    nj2 = J - DRK2
    xp2 = np.zeros((C, DC, 2, nj2, B), dtype=F8E4)
    xp2[:, :, 0] = xq[:, :, :nj2]
    xp2[:, :, 1, 1:] = xq[:, :, :nj2 - 1]
    return {"xt": xt, "ft": ft, "ft16": ft16,
            "ftp": np.ascontiguousarray(ftp), "ftp2": np.ascontiguousarray(ftp2),
            "xp": xp.reshape(C, DC, 2, NP), "xp2": xp2.reshape(C, DC, 2, NP2)}


def _run(x, f, trace=False):
    from concourse.bass_utils import run_bass_kernel_spmd

    nc = _build_nc()
    in_maps = [_prep_core_inputs(x, f, i) for i in range(NCORES)]
    res = run_bass_kernel_spmd(
        nc, in_maps, core_ids=list(range(NCORES)), trace=trace
    )

    y = np.empty((B, D, L), dtype=np.float32)
    for i in range(NCORES):
        ytc = np.asarray(res.results[i]["yt"]).astype(np.float32)  # [C(a), DC, N]
        ys = ytc.reshape(C, DC, J, B).transpose(3, 1, 2, 0).reshape(B, DC, L)
        y[:, i * DC:(i + 1) * DC, :] = ys
    return y, res


def kernel(x, filter):
    x = np.asarray(x, dtype=np.float32)
    f = np.asarray(filter, dtype=np.float32)
    y, _ = _run(x, f, trace=False)
    return y
